# revision 12
# baseline (speedup 1.0000x reference)
"""Masked multi-head attention kernel for 8 Trainium2 NeuronCores.

Strategy (v2):
  - 24 (batch, head) pairs sharded as: core c -> batch c//4, heads [3*(c%4) .. 3*(c%4)+2].
  - Key-padding mask handled by HOST-side gather: only unmasked key positions are
    shipped/computed. Padded key slots get zeroed K columns (scores=0 -> exp=1)
    and a 0 in the indicator column of V, so they contribute nothing.
  - Softmax without max-subtraction; row-sum of exp folded into the AV matmul via
    indicator columns on V (cols 64:128; V values in cols 0:64 so O lands on psum
    partitions 0:64 and the denominator on 64:128).
  - exp split column-wise between ScalarE (true exp -> bf16, queries 0:CUT of each
    1024-query unit) and VectorE (Schraudolph bitcast exp: int16(s*A+B) read as
    bf16, queries CUT:1024) so score-tile drain keeps pace with PE production.
  - Output projection packs heads 0,1 on a 128-partition contraction (OTp), head 2
    separate (OT2); per-core partial written as bf16, host sums 4 partials/batch.
  - QKV accumulation is t-outer so matmuls start as soon as each 128-row
    contraction tile of x arrives from HBM (keeps PE busy during input DMA).
  - All matmuls bf16, fp32 PSUM accumulation (fp8 would blow the error budget).
"""

import math

import numpy as np
import ml_dtypes

BF16 = ml_dtypes.bfloat16
B, N, C = 2, 2048, 768
H = 12
D = 64
HPC = 3          # heads per core
P = 128
SCALE = D ** -0.5
NCORES = 8
CUT = 640        # queries per 1024 handled by ScalarE exp; rest by VectorE
LOG2E = 1.4426950408889634
SCH_A = 128.0 * LOG2E * SCALE
SCH_B = 127.0 * 128.0 - 4.5
_DEBUG = False  # adds intermediate-dump DRAM outputs (devloop only)


def _build_program(KP: int):
    from concourse import bacc, mybir
    from concourse.tile import TileContext

    JG = KP // P
    f32 = mybir.dt.float32
    bf16 = mybir.dt.bfloat16
    i16 = mybir.dt.int16
    Alu = mybir.AluOpType
    nc = bacc.Bacc(None, target_bir_lowering=False)

    xT_d = nc.declare_dram_parameter("xT", [P, 6, N], bf16, False)
    xTk_d = nc.declare_dram_parameter("xTk", [P, 6, KP], bf16, False)
    kf_d = nc.declare_dram_parameter("kf", [P, JG, HPC, D], bf16, False)
    wq_d = nc.declare_dram_parameter("wqT", [P, 6, 192], bf16, False)
    wk_d = nc.declare_dram_parameter("wkT", [P, 6, 192], bf16, False)
    wv_d = nc.declare_dram_parameter("wvT", [P, 6, 192], bf16, False)
    p01_d = nc.declare_dram_parameter("pT01", [P, 6, P], bf16, False)
    p2_d = nc.declare_dram_parameter("pT2", [D, 6, P], bf16, False)
    out_d = nc.declare_dram_parameter("outT", [P, 6, N], bf16, True)
    if _DEBUG:
        dbg = {
            "dbg_q01": nc.declare_dram_parameter("dbg_q01", [P, N], bf16, True),
            "dbg_q2": nc.declare_dram_parameter("dbg_q2", [D, N], bf16, True),
            "dbg_k01": nc.declare_dram_parameter("dbg_k01", [P, KP], bf16, True),
            "dbg_k2": nc.declare_dram_parameter("dbg_k2", [D, KP], bf16, True),
            "dbg_v": nc.declare_dram_parameter("dbg_v", [P, JG, HPC, 2 * D], bf16, True),
            "dbg_pt": nc.declare_dram_parameter("dbg_pt", [P, 1024], bf16, True),
            "dbg_otp": nc.declare_dram_parameter("dbg_otp", [P, N], bf16, True),
            "dbg_po": nc.declare_dram_parameter("dbg_po", [P, 1024], f32, True),
            "dbg_rb": nc.declare_dram_parameter("dbg_rb", [D, 1024], f32, True),
            "dbg_ot2": nc.declare_dram_parameter("dbg_ot2", [D, N], bf16, True),
        }

    with TileContext(nc) as tc:
        with (
            tc.tile_pool(name="const", bufs=1) as cpool,
            tc.tile_pool(name="pt", bufs=4) as ptpool,
            tc.tile_pool(name="rb", bufs=2) as rbpool,
            tc.tile_pool(name="outp", bufs=3) as opool,
            tc.tile_pool(name="ps", bufs=2, space="PSUM") as pspool,
            tc.tile_pool(name="po", bufs=2, space="PSUM") as popool,
        ):
            # ---- persistent SBUF tiles
            wq = cpool.tile([P, 6, 192], bf16)
            wk = cpool.tile([P, 6, 192], bf16)
            wv = cpool.tile([P, 6, 192], bf16)
            xTt = [cpool.tile([P, N], bf16, name=f"xTt{t}") for t in range(6)]
            xkt = [cpool.tile([P, KP], bf16, name=f"xkt{t}") for t in range(6)]
            p01 = cpool.tile([P, 6, P], bf16)
            p2 = cpool.tile([D, 6, P], bf16)
            qT01 = cpool.tile([P, N], bf16)
            qT2 = cpool.tile([D, N], bf16)
            kT01 = cpool.tile([P, KP], bf16)
            kT2 = cpool.tile([D, KP], bf16)
            v_sb = cpool.tile([P, JG, HPC, 2 * D], bf16)
            OTp = cpool.tile([P, N], bf16)
            OT2 = cpool.tile([D, N], bf16)
            if _DEBUG:
                dbg_sb1 = cpool.tile([P, 1024], f32)
                dbg_sb2 = cpool.tile([P, 1024], f32)

            # ---- input DMA: per-t tiles so matmuls start as each 128-row
            # contraction tile lands; xTk rides the Activation DMA queue in
            # parallel with xT on the sync queue.
            nc.sync.dma_start(wq[:], wq_d[:])
            nc.scalar.dma_start(wk[:], wk_d[:])
            nc.scalar.dma_start(wv[:], wv_d[:])
            for t in range(6):
                nc.sync.dma_start(xTt[t][:], xT_d[:, t, :])
                nc.scalar.dma_start(xkt[t][:], xTk_d[:, t, :])
            # indicator columns of V straight from DRAM (cols 0:64 ->
            # denominator lands on psum partitions 0:64, proven recip pattern)
            nc.sync.dma_start(v_sb[:, :, :, 0:D], kf_d[:])
            nc.sync.dma_start(p01[:], p01_d[:])
            nc.sync.dma_start(p2[:], p2_d[:])

            NKC = (KP + 1023) // 1024  # k-chain 1024-col chunks (1 for KP=1024)

            # ---- q01 (heads 0,1 -> 128 rows), k01, k2: one t-outer loop,
            # matmuls chase the incoming DMA tiles (8 banks of PSUM in play)
            psq = [popool.tile([P, 1024], f32, tag="po", name=f"psq{i}") for i in range(2)]
            psk = [pspool.tile([P, 1024], f32, tag="ps", name=f"psk{i}") for i in range(NKC)]
            # k2 shares the t-loop only when both k psums fit the pool (NKC==1)
            if NKC == 1:
                psk2 = [pspool.tile([P, 1024], f32, tag="ps", name=f"psk2_{i}") for i in range(NKC)]
            for t in range(6):
                for half in range(2):
                    for s0 in (0, 512):
                        nc.tensor.matmul(
                            psq[half][:, s0 : s0 + 512],
                            wq[:, t, 0:P],
                            xTt[t][:, half * 1024 + s0 : half * 1024 + s0 + 512],
                            start=(t == 0),
                            stop=(t == 5),
                        )
                for kc in range(NKC):
                    for s0 in range(0, min(1024, KP - kc * 1024), 512):
                        c0 = kc * 1024 + s0
                        csz = min(512, KP - c0)
                        nc.tensor.matmul(
                            psk[kc][:, s0 : s0 + csz],
                            wk[:, t, 0:P],
                            xkt[t][:, c0 : c0 + csz],
                            start=(t == 0),
                            stop=(t == 5),
                        )
                        if NKC == 1:
                            nc.tensor.matmul(
                                psk2[kc][:D, s0 : s0 + csz],
                                wk[:, t, P:192],
                                xkt[t][:, c0 : c0 + csz],
                                start=(t == 0),
                                stop=(t == 5),
                            )
            for half in range(2):
                nc.scalar.copy(qT01[:, half * 1024 : half * 1024 + 1024], psq[half][:])
            for kc in range(NKC):
                csz = min(1024, KP - kc * 1024)
                nc.scalar.copy(kT01[:, kc * 1024 : kc * 1024 + csz], psk[kc][:, :csz])
            if NKC == 1:
                nc.scalar.copy(kT2[:, :KP], psk2[0][:D, :KP])
            else:
                psk2 = [pspool.tile([P, 1024], f32, tag="ps", name=f"psk2b_{i}") for i in range(NKC)]
                for t in range(6):
                    for kc in range(NKC):
                        for s0 in range(0, min(1024, KP - kc * 1024), 512):
                            c0 = kc * 1024 + s0
                            csz = min(512, KP - c0)
                            nc.tensor.matmul(
                                psk2[kc][:D, s0 : s0 + csz],
                                wk[:, t, P:192],
                                xkt[t][:, c0 : c0 + csz],
                                start=(t == 0),
                                stop=(t == 5),
                            )
                for kc in range(NKC):
                    csz = min(1024, KP - kc * 1024)
                    nc.scalar.copy(kT2[:, kc * 1024 : kc * 1024 + csz], psk2[kc][:D, :csz])

            for jg in range(JG):
                psv = pspool.tile([P, 1024], f32, tag="ps")
                for t in range(6):
                    nc.tensor.matmul(
                        psv[:, :192],
                        xkt[t][:, jg * P : (jg + 1) * P],
                        wv[:, t, :],
                        start=(t == 0),
                        stop=(t == 5),
                    )
                for ih in range(HPC):
                    nc.vector.tensor_copy(
                        v_sb[:, jg, ih, D : 2 * D], psv[:, ih * D : (ih + 1) * D]
                    )

            psq2 = [popool.tile([P, 1024], f32, tag="po", name=f"psq2_{i}") for i in range(2)]
            for t in range(6):
                for half in range(2):
                    for s0 in (0, 512):
                        nc.tensor.matmul(
                            psq2[half][:D, s0 : s0 + 512],
                            wq[:, t, P:192],
                            xTt[t][:, half * 1024 + s0 : half * 1024 + s0 + 512],
                            start=(t == 0),
                            stop=(t == 5),
                        )
            for half in range(2):
                nc.scalar.copy(qT2[:, half * 1024 : half * 1024 + 1024], psq2[half][:D, :])

            # ---- attention units: (head, query-half), 1024 queries each.
            # Units run in interleaved PAIRS so each unit's exp latency hides
            # behind the other unit's scores/AV matmuls and PE stays gapless.
            def att_pair(pair):
                states = []
                for (ih, nh) in pair:
                    po_t = popool.tile(
                        [P, 1024], f32, tag="po", name=f"po_{ih}_{nh}"
                    )
                    states.append({"ih": ih, "nh": nh, "po": po_t, "pts": [None] * JG})

                def scores(st, jg):
                    ih, nh = st["ih"], st["nh"]
                    q0 = nh * 1024
                    qT_h = qT01[D * ih : D * (ih + 1), :] if ih < 2 else qT2[:, :]
                    kT_h = kT01[D * ih : D * (ih + 1), :] if ih < 2 else kT2[:, :]
                    psc = pspool.tile([P, 1024], f32, tag="ps", name=f"psc{ih}{nh}{jg}")
                    for s0 in (0, 512):
                        nc.tensor.matmul(
                            psc[:, s0 : s0 + 512],
                            kT_h[:, jg * P : (jg + 1) * P],
                            qT_h[:, q0 + s0 : q0 + s0 + 512],
                            start=True,
                            stop=True,
                        )
                    pt = ptpool.tile([P, 1024], bf16, tag="pt", name=f"pt{ih}{nh}{jg}")
                    nc.scalar.activation(
                        pt[:, 0:CUT],
                        psc[:, 0:CUT],
                        mybir.ActivationFunctionType.Exp,
                        scale=float(SCALE),
                    )
                    nc.vector.tensor_scalar(
                        pt[:, CUT:1024].bitcast(i16),
                        psc[:, CUT:1024],
                        float(SCH_A),
                        float(SCH_B),
                        Alu.mult,
                        Alu.add,
                    )
                    if _DEBUG and ih == 0 and nh == 0 and jg == 0:
                        nc.sync.dma_start(dbg["dbg_pt"][:], pt[:])
                    st["pts"][jg] = pt

                def av(st, jg):
                    ih = st["ih"]
                    pt = st["pts"][jg]
                    for s0 in (0, 512):
                        nc.tensor.matmul(
                            st["po"][:, s0 : s0 + 512],
                            v_sb[:, jg, ih, :],
                            pt[:, s0 : s0 + 512],
                            start=(jg == 0),
                            stop=(jg == JG - 1),
                        )
                    st["pts"][jg] = None

                for jg in range(JG):
                    for st in states:
                        scores(st, jg)
                    if jg >= 1:
                        for st in states:
                            av(st, jg - 1)
                for st in states:
                    av(st, JG - 1)

                for st in states:
                    ih, nh, po_t = st["ih"], st["nh"], st["po"]
                    q0 = nh * 1024
                    rb_t = rbpool.tile([P, 1024], f32, tag="rb", name=f"rb{ih}{nh}")
                    nc.vector.reciprocal_approx_fast(rb_t[0:D, :], po_t[0:D, :])
                    if ih == 0:
                        ot_dst = OTp[0:D, q0 : q0 + 1024]
                    elif ih == 1:
                        ot_dst = OTp[D : 2 * D, q0 : q0 + 1024]
                    else:
                        ot_dst = OT2[:, q0 : q0 + 1024]
                    nc.vector.tensor_mul(ot_dst, po_t[D : 2 * D, :], rb_t[0:D, :])
                    if _DEBUG and ih == 0 and nh == 0:
                        nc.scalar.copy(dbg_sb1[:], po_t[:])
                        nc.sync.dma_start(dbg["dbg_po"][:], dbg_sb1[:])
                        nc.vector.tensor_copy(dbg_sb2[0:D, :], rb_t[0:D, :])
                        nc.sync.dma_start(dbg["dbg_rb"][:], dbg_sb2[0:D, :])

            # ---- partial projection for one query half (6 column groups)
            def proj(nh):
                q0 = nh * 1024
                for cg in range(6):
                    pp = pspool.tile([P, 1024], f32, tag="ps")
                    for s0 in (0, 512):
                        nc.tensor.matmul(
                            pp[:, s0 : s0 + 512],
                            p01[:, cg, :],
                            OTp[:, q0 + s0 : q0 + s0 + 512],
                            start=True,
                            stop=False,
                        )
                        nc.tensor.matmul(
                            pp[:, s0 : s0 + 512],
                            p2[:, cg, :],
                            OT2[:, q0 + s0 : q0 + s0 + 512],
                            start=False,
                            stop=True,
                        )
                    ob = opool.tile([P, 1024], bf16)
                    if cg % 2 == 0:
                        nc.vector.tensor_copy(ob[:], pp[:])
                    else:
                        nc.scalar.copy(ob[:], pp[:])
                    nc.sync.dma_start(out_d[:, cg, q0 : q0 + 1024], ob[:])

            att_pair([(0, 0), (1, 0)])
            att_pair([(2, 0), (0, 1)])
            proj(0)
            att_pair([(1, 1), (2, 1)])
            proj(1)

            if _DEBUG:
                nc.sync.dma_start(dbg["dbg_q01"][:], qT01[:])
                nc.sync.dma_start(dbg["dbg_q2"][:], qT2[:])
                nc.sync.dma_start(dbg["dbg_k01"][:], kT01[:])
                nc.sync.dma_start(dbg["dbg_k2"][:], kT2[:])
                nc.sync.dma_start(dbg["dbg_v"][:], v_sb[:])
                nc.sync.dma_start(dbg["dbg_otp"][:], OTp[:])
                nc.sync.dma_start(dbg["dbg_ot2"][:], OT2[:])

    nc.finalize()
    return nc


def _prep_inputs(x, mask, qkv_w, proj_w):
    """Build the 8 per-core input maps. Returns (in_maps, KP)."""
    idx = [np.nonzero(mask[b] == 0.0)[0] for b in range(B)]
    nk = max(len(i) for i in idx)
    KP = max(P, int(math.ceil(nk / P)) * P)
    JG = KP // P

    per_batch = []
    for b in range(B):
        xTb = np.ascontiguousarray(x[b].T)  # [C, N] f32
        xT_in = xTb.reshape(6, P, N).transpose(1, 0, 2).astype(BF16)
        xk = np.zeros((C, KP), np.float32)
        xk[:, : len(idx[b])] = xTb[:, idx[b]]
        xTk_in = xk.reshape(6, P, KP).transpose(1, 0, 2).astype(BF16)
        kfv = np.zeros((KP,), np.float32)
        kfv[: len(idx[b])] = 1.0
        kf_in = np.ascontiguousarray(
            np.broadcast_to(
                kfv.reshape(JG, P).T[:, :, None, None], (P, JG, HPC, D)
            )
        ).astype(BF16)
        per_batch.append((xT_in, xTk_in, kf_in))

    in_maps = []
    for c in range(NCORES):
        b, g = c // 4, c % 4
        h0 = HPC * g
        xT_in, xTk_in, kf_in = per_batch[b]
        m = {"xT": xT_in, "xTk": xTk_in, "kf": kf_in}
        for name, off in (("wqT", 0), ("wkT", C), ("wvT", 2 * C)):
            w = qkv_w[off + h0 * D : off + (h0 + HPC) * D]  # [192, C]
            m[name] = (
                np.ascontiguousarray(w.T).reshape(6, P, 192).transpose(1, 0, 2).astype(BF16)
            )
        m["pT01"] = np.ascontiguousarray(
            proj_w[:, h0 * D : (h0 + 2) * D].T
        ).reshape(P, 6, P).astype(BF16)
        m["pT2"] = np.ascontiguousarray(
            proj_w[:, (h0 + 2) * D : (h0 + 3) * D].T
        ).reshape(D, 6, P).astype(BF16)
        in_maps.append(m)
    return in_maps, KP


_CACHE = {}


def _get_program(KP):
    if KP not in _CACHE:
        _CACHE[KP] = _build_program(KP)
    return _CACHE[KP]


def kernel(x, mask, qkv_w, proj_w, proj_b, _want_results=False):
    from concourse.bass_utils import run_bass_kernel_spmd

    x = np.asarray(x, np.float32)
    mask = np.asarray(mask, np.float32)
    qkv_w = np.asarray(qkv_w, np.float32)
    proj_w = np.asarray(proj_w, np.float32)
    proj_b = np.asarray(proj_b, np.float32)

    in_maps, KP = _prep_inputs(x, mask, qkv_w, proj_w)
    nc = _get_program(KP)
    res = run_bass_kernel_spmd(nc, in_maps, list(range(NCORES)))

    out = np.empty((B, N, C), np.float32)
    for b in range(B):
        acc = None
        for c in range(4 * b, 4 * b + 4):
            a = res.results[c]["outT"]  # [128, 6, N] bf16
            a = np.asarray(a, np.float32).transpose(1, 0, 2).reshape(C, N)
            acc = a if acc is None else acc + a
        out[b] = acc.T + proj_b[None, :]
    if _want_results:
        return out, res
    return out


# revision 14
# speedup vs baseline: 1.0577x; 1.0577x over previous
"""Masked multi-head attention kernel for 8 Trainium2 NeuronCores.

Strategy (v4):
  - 24 (batch, head) pairs sharded as: core c -> batch c//4, heads [3*(c%4) .. 3*(c%4)+2].
  - Key-padding mask handled by HOST-side gather: only unmasked key positions are
    shipped/computed. Padded key slots get zeroed K columns (scores=0 -> exp=1)
    and a 0 in the indicator column of V, so they contribute nothing.
  - Softmax without max-subtraction; row-sum of exp folded into the AV matmul via
    indicator columns on V (cols 0:64 -> denominator on psum partitions 0:64,
    V values in cols 64:128 -> O on partitions 64:128).
  - exp split by 512-query chunk: chunk A -> ScalarE true exp (bf16), chunk B ->
    VectorE Schraudolph bitcast exp (int16(s*A+B) reinterpreted as bf16), so the
    two engines drain score tiles in parallel faster than PE refills them.
    First jg of each unit runs fully on ScalarE to absorb VectorE's recip/mul
    burst from the previous unit.
  - Phase-scoped PSUM pools: A1 (k01+k2+v, t-outer over arriving xTk tiles),
    A2 (q01+q2 t-outer over arriving xT tiles, 8x 1-bank slots), B/C (4 score
    slots + 4 AV-accumulator slots, all [128,512] single-bank).
  - Output projection packs heads 0,1 on a 128-partition contraction (OTp),
    head 2 separate (OT2); per-core partial written as bf16, host sums 4
    partials per batch. proj(0) split across two insertion points to spread
    its PSUM-drain copies.
  - All matmuls bf16, fp32 PSUM accumulation (fp8 blows the 2e-2 error budget).
"""

import math

import numpy as np
import ml_dtypes

BF16 = ml_dtypes.bfloat16
B, N, C = 2, 2048, 768
H = 12
D = 64
HPC = 3          # heads per core
P = 128
SCALE = D ** -0.5
NCORES = 8
LOG2E = 1.4426950408889634
SCH_A = 128.0 * LOG2E * SCALE
SCH_B = 127.0 * 128.0 - 4.5
_DEBUG = False


def _build_program(KP: int):
    from concourse import bacc, mybir
    from concourse.tile import TileContext

    JG = KP // P
    f32 = mybir.dt.float32
    bf16 = mybir.dt.bfloat16
    i16 = mybir.dt.int16
    Alu = mybir.AluOpType
    Exp = mybir.ActivationFunctionType.Exp
    nc = bacc.Bacc(None, target_bir_lowering=False)

    xT_d = nc.declare_dram_parameter("xT", [P, 6, N], bf16, False)
    xTk_d = nc.declare_dram_parameter("xTk", [P, 6, KP], bf16, False)
    kf_d = nc.declare_dram_parameter("kf", [P, JG, HPC, D], bf16, False)
    wq_d = nc.declare_dram_parameter("wqT", [P, 6, 192], bf16, False)
    wk_d = nc.declare_dram_parameter("wkT", [P, 6, 192], bf16, False)
    wv_d = nc.declare_dram_parameter("wvT", [P, 6, 192], bf16, False)
    p01_d = nc.declare_dram_parameter("pT01", [P, 6, P], bf16, False)
    p2_d = nc.declare_dram_parameter("pT2", [D, 6, P], bf16, False)
    out_d = nc.declare_dram_parameter("outT", [P, 6, N], bf16, True)

    fastA = KP <= 1024

    with TileContext(nc) as tc:
        with (
            tc.tile_pool(name="const", bufs=1) as cpool,
            tc.tile_pool(name="pt", bufs=6) as ptpool,
            tc.tile_pool(name="rb", bufs=4) as rbpool,
            tc.tile_pool(name="outp", bufs=4) as opool,
        ):
            # ---- persistent SBUF tiles
            wq = cpool.tile([P, 6, 192], bf16)
            wk = cpool.tile([P, 6, 192], bf16)
            wv = cpool.tile([P, 6, 192], bf16)
            xTt = [cpool.tile([P, N], bf16, name=f"xTt{t}") for t in range(6)]
            xkt = [cpool.tile([P, KP], bf16, name=f"xkt{t}") for t in range(6)]
            p01 = cpool.tile([P, 6, P], bf16)
            p2 = cpool.tile([D, 6, P], bf16)
            qT01 = cpool.tile([P, N], bf16)
            qT2 = cpool.tile([D, N], bf16)
            kT01 = cpool.tile([P, KP], bf16)
            kT2 = cpool.tile([D, KP], bf16)
            v_sb = cpool.tile([P, JG, HPC, 2 * D], bf16)
            OTp = cpool.tile([P, N], bf16)
            OT2 = cpool.tile([D, N], bf16)
            if _DEBUG:
                dbg = {
                    "dbg_pt": nc.declare_dram_parameter("dbg_pt", [P, 512], bf16, True),
                    "dbg_po": nc.declare_dram_parameter("dbg_po", [P, 512], f32, True),
                }
                dbg_sb1 = cpool.tile([P, 512], f32)

            # ---- input DMA. xTk first (k/v work starts earliest), then xT.
            # Two queues (sync + scalar engine) split the streams.
            nc.sync.dma_start(wk[:], wk_d[:])
            nc.scalar.dma_start(wv[:], wv_d[:])
            nc.scalar.dma_start(v_sb[:, :, :, 0:D], kf_d[:])
            for t in range(3):
                nc.sync.dma_start(xkt[t][:], xTk_d[:, t, :])
                nc.scalar.dma_start(xkt[t + 3][:], xTk_d[:, t + 3, :])
            nc.sync.dma_start(wq[:], wq_d[:])
            nc.scalar.dma_start(p01[:], p01_d[:])
            nc.scalar.dma_start(p2[:], p2_d[:])
            for t in range(3):
                nc.sync.dma_start(xTt[t][:], xT_d[:, t, :])
                nc.scalar.dma_start(xTt[t + 3][:], xT_d[:, t + 3, :])

            # ---- A1: k01, k2, v — t-outer over arriving xTk tiles
            if fastA:
                with tc.tile_pool(name="pa", bufs=1, space="PSUM") as pa:
                    psk = pa.tile([P, KP], f32, tag="psk")
                    psk2 = pa.tile([P, KP], f32, tag="psk2")
                    # v runs in waves of 4 key groups; each group's accumulator
                    # gets its own full PSUM bank (512 f32) so concurrent
                    # accumulation groups never share a zero region.
                    for w0 in range(0, JG, 4):
                        nw = min(4, JG - w0)
                        psv = pa.tile([P, 4, 512], f32, tag="psv")
                        for t in range(6):
                            if w0 == 0:
                                for s0 in range(0, KP, 512):
                                    csz = min(512, KP - s0)
                                    nc.tensor.matmul(
                                        psk[:, s0 : s0 + csz],
                                        wk[:, t, 0:P],
                                        xkt[t][:, s0 : s0 + csz],
                                        start=(t == 0),
                                        stop=(t == 5),
                                    )
                                    nc.tensor.matmul(
                                        psk2[:D, s0 : s0 + csz],
                                        wk[:, t, P:192],
                                        xkt[t][:, s0 : s0 + csz],
                                        start=(t == 0),
                                        stop=(t == 5),
                                    )
                            for j in range(nw):
                                jg = w0 + j
                                nc.tensor.matmul(
                                    psv[:, j, 0:192],
                                    xkt[t][:, jg * P : (jg + 1) * P],
                                    wv[:, t, :],
                                    start=(t == 0),
                                    stop=(t == 5),
                                )
                        if w0 == 0:
                            nc.scalar.copy(kT01[:], psk[:])
                            nc.scalar.copy(kT2[:], psk2[:D, :])
                        for j in range(nw):
                            jg = w0 + j
                            for ih in range(HPC):
                                nc.vector.tensor_copy(
                                    v_sb[:, jg, ih, D : 2 * D],
                                    psv[:, j, ih * D : (ih + 1) * D],
                                )
            else:
                # generic fallback: sequential chains
                with tc.tile_pool(name="pa", bufs=2, space="PSUM") as pa:
                    for kc in range(0, KP, 1024):
                        csz = min(1024, KP - kc)
                        pk = pa.tile([P, 1024], f32, tag="g", name=f"pk_k01_{kc}")
                        for t in range(6):
                            for s0 in range(0, csz, 512):
                                ssz = min(512, csz - s0)
                                nc.tensor.matmul(
                                    pk[:, s0 : s0 + ssz],
                                    wk[:, t, 0:P],
                                    xkt[t][:, kc + s0 : kc + s0 + ssz],
                                    start=(t == 0),
                                    stop=(t == 5),
                                )
                        nc.scalar.copy(kT01[:, kc : kc + csz], pk[:, :csz])
                    for kc in range(0, KP, 1024):
                        csz = min(1024, KP - kc)
                        pk = pa.tile([P, 1024], f32, tag="g", name=f"pk_k2_{kc}")
                        for t in range(6):
                            for s0 in range(0, csz, 512):
                                ssz = min(512, csz - s0)
                                nc.tensor.matmul(
                                    pk[:D, s0 : s0 + ssz],
                                    wk[:, t, P:192],
                                    xkt[t][:, kc + s0 : kc + s0 + ssz],
                                    start=(t == 0),
                                    stop=(t == 5),
                                )
                        nc.scalar.copy(kT2[:, kc : kc + csz], pk[:D, :csz])
                    for jg in range(JG):
                        pk = pa.tile([P, 1024], f32, tag="g", name=f"pk_v_{jg}")
                        for t in range(6):
                            nc.tensor.matmul(
                                pk[:, :192],
                                xkt[t][:, jg * P : (jg + 1) * P],
                                wv[:, t, :],
                                start=(t == 0),
                                stop=(t == 5),
                            )
                        for ih in range(HPC):
                            nc.vector.tensor_copy(
                                v_sb[:, jg, ih, D : 2 * D],
                                pk[:, ih * D : (ih + 1) * D],
                            )

            # ---- A2: q01 and q2, t-outer over arriving xT tiles
            with tc.tile_pool(name="pa2", bufs=4, space="PSUM") as pa2:
                psq = [pa2.tile([P, 512], f32, tag="qA", name=f"psqA{i}") for i in range(4)]
                psq2 = [pa2.tile([P, 512], f32, tag="qB", name=f"psqB{i}") for i in range(4)]
                for t in range(6):
                    for i in range(4):
                        nc.tensor.matmul(
                            psq[i][:],
                            wq[:, t, 0:P],
                            xTt[t][:, i * 512 : (i + 1) * 512],
                            start=(t == 0),
                            stop=(t == 5),
                        )
                        nc.tensor.matmul(
                            psq2[i][:D, :],
                            wq[:, t, P:192],
                            xTt[t][:, i * 512 : (i + 1) * 512],
                            start=(t == 0),
                            stop=(t == 5),
                        )
                for i in range(4):
                    nc.scalar.copy(qT01[:, i * 512 : (i + 1) * 512], psq[i][:])
                    if i % 2 == 0:
                        nc.scalar.copy(qT2[:, i * 512 : (i + 1) * 512], psq2[i][:D, :])
                    else:
                        nc.vector.tensor_copy(qT2[:, i * 512 : (i + 1) * 512], psq2[i][:D, :])

            # ---- B + C
            with (
                tc.tile_pool(name="ps", bufs=4, space="PSUM") as pspool,
                tc.tile_pool(name="po", bufs=4, space="PSUM") as popool,
            ):
                def att_unit(ih, nh):
                    q0 = nh * 1024
                    qT_h = qT01[D * ih : D * (ih + 1), :] if ih < 2 else qT2[:, :]
                    kT_h = kT01[D * ih : D * (ih + 1), :] if ih < 2 else kT2[:, :]
                    po = [
                        popool.tile([P, 512], f32, tag="po", name=f"po{ih}{nh}{c}")
                        for c in range(2)
                    ]
                    pts = [None] * JG

                    def scores(jg):
                        pair = []
                        for c in range(2):
                            psc = pspool.tile(
                                [P, 512], f32, tag="ps", name=f"psc{ih}{nh}{jg}{c}"
                            )
                            nc.tensor.matmul(
                                psc[:],
                                kT_h[:, jg * P : (jg + 1) * P],
                                qT_h[:, q0 + c * 512 : q0 + (c + 1) * 512],
                                start=True,
                                stop=True,
                            )
                            pair.append(psc)
                        ptp = []
                        for c in range(2):
                            pt = ptpool.tile(
                                [P, 512], bf16, tag="pt", name=f"pt{ih}{nh}{jg}{c}"
                            )
                            if c == 0 or jg == 0:
                                nc.scalar.activation(
                                    pt[:], pair[c][:], Exp, scale=float(SCALE)
                                )
                            else:
                                nc.vector.tensor_scalar(
                                    pt[:].bitcast(i16),
                                    pair[c][:],
                                    float(SCH_A),
                                    float(SCH_B),
                                    Alu.mult,
                                    Alu.add,
                                )
                            ptp.append(pt)
                        if _DEBUG and ih == 0 and nh == 0 and jg == 0:
                            nc.sync.dma_start(dbg["dbg_pt"][:], ptp[0][:])
                        pts[jg] = ptp

                    def av(jg):
                        for c in range(2):
                            nc.tensor.matmul(
                                po[c][:],
                                v_sb[:, jg, ih, :],
                                pts[jg][c][:],
                                start=(jg == 0),
                                stop=(jg == JG - 1),
                            )
                        pts[jg] = None

                    scores(0)
                    for jg in range(1, JG):
                        scores(jg)
                        av(jg - 1)
                    av(JG - 1)

                    if ih == 0:
                        ot_dst = OTp[0:D, :]
                    elif ih == 1:
                        ot_dst = OTp[D : 2 * D, :]
                    else:
                        ot_dst = OT2[:, :]
                    for c in range(2):
                        rb_t = rbpool.tile([P, 512], f32, tag="rb", name=f"rb{ih}{nh}{c}")
                        nc.vector.reciprocal_approx_fast(rb_t[0:D, :], po[c][0:D, :])
                        nc.vector.tensor_mul(
                            ot_dst[:, q0 + c * 512 : q0 + (c + 1) * 512],
                            po[c][D : 2 * D, :],
                            rb_t[0:D, :],
                        )
                        if _DEBUG and ih == 0 and nh == 0 and c == 0:
                            nc.scalar.copy(dbg_sb1[:], po[c][:])
                            nc.sync.dma_start(dbg["dbg_po"][:], dbg_sb1[:])

                def proj(nh, cgs):
                    q0 = nh * 1024
                    for cg in cgs:
                        for c in range(2):
                            pp = pspool.tile(
                                [P, 512], f32, tag="ps", name=f"pp{nh}{cg}{c}"
                            )
                            s0 = q0 + c * 512
                            nc.tensor.matmul(
                                pp[:],
                                p01[:, cg, :],
                                OTp[:, s0 : s0 + 512],
                                start=True,
                                stop=False,
                            )
                            nc.tensor.matmul(
                                pp[:],
                                p2[:, cg, :],
                                OT2[:, s0 : s0 + 512],
                                start=False,
                                stop=True,
                            )
                            ob = opool.tile([P, 512], bf16, name=f"ob{nh}{cg}{c}")
                            if (cg + c) % 2 == 0:
                                nc.vector.tensor_copy(ob[:], pp[:])
                            else:
                                nc.scalar.copy(ob[:], pp[:])
                            nc.sync.dma_start(
                                out_d[:, cg, s0 : s0 + 512], ob[:]
                            )

                att_unit(0, 0)
                att_unit(1, 0)
                att_unit(2, 0)
                proj(0, range(0, 3))
                att_unit(0, 1)
                proj(0, range(3, 6))
                att_unit(1, 1)
                att_unit(2, 1)
                proj(1, range(0, 6))

    nc.finalize()
    return nc


def _prep_inputs(x, mask, qkv_w, proj_w):
    """Build the 8 per-core input maps. Returns (in_maps, KP)."""
    idx = [np.nonzero(mask[b] == 0.0)[0] for b in range(B)]
    nk = max(len(i) for i in idx)
    KP = max(P, int(math.ceil(nk / P)) * P)
    JG = KP // P

    per_batch = []
    for b in range(B):
        xTb = np.ascontiguousarray(x[b].T)  # [C, N] f32
        xT_in = xTb.reshape(6, P, N).transpose(1, 0, 2).astype(BF16)
        xk = np.zeros((C, KP), np.float32)
        xk[:, : len(idx[b])] = xTb[:, idx[b]]
        xTk_in = xk.reshape(6, P, KP).transpose(1, 0, 2).astype(BF16)
        kfv = np.zeros((KP,), np.float32)
        kfv[: len(idx[b])] = 1.0
        kf_in = np.ascontiguousarray(
            np.broadcast_to(
                kfv.reshape(JG, P).T[:, :, None, None], (P, JG, HPC, D)
            )
        ).astype(BF16)
        per_batch.append((xT_in, xTk_in, kf_in))

    in_maps = []
    for c in range(NCORES):
        b, g = c // 4, c % 4
        h0 = HPC * g
        xT_in, xTk_in, kf_in = per_batch[b]
        m = {"xT": xT_in, "xTk": xTk_in, "kf": kf_in}
        for name, off in (("wqT", 0), ("wkT", C), ("wvT", 2 * C)):
            w = qkv_w[off + h0 * D : off + (h0 + HPC) * D]  # [192, C]
            m[name] = (
                np.ascontiguousarray(w.T).reshape(6, P, 192).transpose(1, 0, 2).astype(BF16)
            )
        m["pT01"] = np.ascontiguousarray(
            proj_w[:, h0 * D : (h0 + 2) * D].T
        ).reshape(P, 6, P).astype(BF16)
        m["pT2"] = np.ascontiguousarray(
            proj_w[:, (h0 + 2) * D : (h0 + 3) * D].T
        ).reshape(D, 6, P).astype(BF16)
        in_maps.append(m)
    return in_maps, KP


_CACHE = {}


def _get_program(KP):
    if KP not in _CACHE:
        _CACHE[KP] = _build_program(KP)
    return _CACHE[KP]


def kernel(x, mask, qkv_w, proj_w, proj_b, _want_results=False):
    from concourse.bass_utils import run_bass_kernel_spmd

    x = np.asarray(x, np.float32)
    mask = np.asarray(mask, np.float32)
    qkv_w = np.asarray(qkv_w, np.float32)
    proj_w = np.asarray(proj_w, np.float32)
    proj_b = np.asarray(proj_b, np.float32)

    in_maps, KP = _prep_inputs(x, mask, qkv_w, proj_w)
    nc = _get_program(KP)
    res = run_bass_kernel_spmd(nc, in_maps, list(range(NCORES)))

    out = np.empty((B, N, C), np.float32)
    for b in range(B):
        acc = None
        for c in range(4 * b, 4 * b + 4):
            a = res.results[c]["outT"]  # [128, 6, N] bf16
            a = np.asarray(a, np.float32).transpose(1, 0, 2).reshape(C, N)
            acc = a if acc is None else acc + a
        out[b] = acc.T + proj_b[None, :]
    if _want_results:
        return out, res
    return out


# revision 15
# speedup vs baseline: 1.0875x; 1.0282x over previous
"""Masked multi-head attention kernel for 8 Trainium2 NeuronCores.

Strategy (v4):
  - 24 (batch, head) pairs sharded as: core c -> batch c//4, heads [3*(c%4) .. 3*(c%4)+2].
  - Key-padding mask handled by HOST-side gather: only unmasked key positions are
    shipped/computed. Padded key slots get zeroed K columns (scores=0 -> exp=1)
    and a 0 in the indicator column of V, so they contribute nothing.
  - Softmax without max-subtraction; row-sum of exp folded into the AV matmul via
    indicator columns on V (cols 0:64 -> denominator on psum partitions 0:64,
    V values in cols 64:128 -> O on partitions 64:128).
  - exp split by 512-query chunk: chunk A -> ScalarE true exp (bf16), chunk B ->
    VectorE Schraudolph bitcast exp (int16(s*A+B) reinterpreted as bf16), so the
    two engines drain score tiles in parallel faster than PE refills them.
    First jg of each unit runs fully on ScalarE to absorb VectorE's recip/mul
    burst from the previous unit.
  - Phase-scoped PSUM pools: A1 (k01+k2+v, t-outer over arriving xTk tiles),
    A2 (q01+q2 t-outer over arriving xT tiles, 8x 1-bank slots), B/C (4 score
    slots + 4 AV-accumulator slots, all [128,512] single-bank).
  - Output projection packs heads 0,1 on a 128-partition contraction (OTp),
    head 2 separate (OT2); per-core partial written as bf16, host sums 4
    partials per batch. proj(0) split across two insertion points to spread
    its PSUM-drain copies.
  - All matmuls bf16, fp32 PSUM accumulation (fp8 blows the 2e-2 error budget).
"""

import math

import numpy as np
import ml_dtypes

BF16 = ml_dtypes.bfloat16
B, N, C = 2, 2048, 768
H = 12
D = 64
HPC = 3          # heads per core
P = 128
SCALE = D ** -0.5
NCORES = 8
LOG2E = 1.4426950408889634
SCH_A = 128.0 * LOG2E * SCALE
SCH_B = 127.0 * 128.0 - 4.5
_DEBUG = False


def _build_program(KP: int):
    from concourse import bacc, mybir
    from concourse.tile import TileContext

    JG = KP // P
    f32 = mybir.dt.float32
    bf16 = mybir.dt.bfloat16
    i16 = mybir.dt.int16
    Alu = mybir.AluOpType
    Exp = mybir.ActivationFunctionType.Exp
    nc = bacc.Bacc(None, target_bir_lowering=False)

    xT_d = nc.declare_dram_parameter("xT", [P, 6, N], bf16, False)
    xTk_d = nc.declare_dram_parameter("xTk", [P, 6, KP], bf16, False)
    kf_d = nc.declare_dram_parameter("kf", [P, JG, HPC, D], bf16, False)
    wq_d = nc.declare_dram_parameter("wqT", [P, 6, 192], bf16, False)
    wk_d = nc.declare_dram_parameter("wkT", [P, 6, 192], bf16, False)
    wv_d = nc.declare_dram_parameter("wvT", [P, 6, 192], bf16, False)
    p01_d = nc.declare_dram_parameter("pT01", [P, 6, P], bf16, False)
    p2_d = nc.declare_dram_parameter("pT2", [D, 6, P], bf16, False)
    out_d = nc.declare_dram_parameter("outT", [P, 6, N], bf16, True)

    fastA = KP <= 1024

    with TileContext(nc) as tc:
        with (
            tc.tile_pool(name="const", bufs=1) as cpool,
            tc.tile_pool(name="pt", bufs=6) as ptpool,
            tc.tile_pool(name="rb", bufs=4) as rbpool,
            tc.tile_pool(name="outp", bufs=4) as opool,
        ):
            # ---- persistent SBUF tiles
            wq = cpool.tile([P, 6, 192], bf16)
            wk = cpool.tile([P, 6, 192], bf16)
            wv = cpool.tile([P, 6, 192], bf16)
            xTt = [cpool.tile([P, N], bf16, name=f"xTt{t}") for t in range(6)]
            xkt = [cpool.tile([P, KP], bf16, name=f"xkt{t}") for t in range(6)]
            p01 = cpool.tile([P, 6, P], bf16)
            p2 = cpool.tile([D, 6, P], bf16)
            qT01 = cpool.tile([P, N], bf16)
            qT2 = cpool.tile([D, N], bf16)
            kT01 = cpool.tile([P, KP], bf16)
            kT2 = cpool.tile([D, KP], bf16)
            v_sb = cpool.tile([P, JG, HPC, 2 * D], bf16)
            OTp = cpool.tile([P, N], bf16)
            OT2 = cpool.tile([D, N], bf16)
            if _DEBUG:
                dbg = {
                    "dbg_pt": nc.declare_dram_parameter("dbg_pt", [P, 512], bf16, True),
                    "dbg_po": nc.declare_dram_parameter("dbg_po", [P, 512], f32, True),
                }
                dbg_sb1 = cpool.tile([P, 512], f32)

            # ---- input DMA. xTk first (k/v work starts earliest), then xT.
            # Two queues (sync + scalar engine) split the streams.
            nc.sync.dma_start(wk[:], wk_d[:])
            nc.scalar.dma_start(wv[:], wv_d[:])
            nc.scalar.dma_start(v_sb[:, :, :, 0:D], kf_d[:])
            for t in range(3):
                nc.sync.dma_start(xkt[t][:], xTk_d[:, t, :])
                nc.scalar.dma_start(xkt[t + 3][:], xTk_d[:, t + 3, :])
            nc.sync.dma_start(wq[:], wq_d[:])
            nc.scalar.dma_start(p01[:], p01_d[:])
            nc.scalar.dma_start(p2[:], p2_d[:])
            for t in range(3):
                nc.sync.dma_start(xTt[t][:], xT_d[:, t, :])
                nc.scalar.dma_start(xTt[t + 3][:], xT_d[:, t + 3, :])

            # ---- A1: k01, k2, v — t-outer over arriving xTk tiles
            if fastA:
                with tc.tile_pool(name="pa", bufs=1, space="PSUM") as pa:
                    psk = pa.tile([P, KP], f32, tag="psk")
                    psk2 = pa.tile([P, KP], f32, tag="psk2")
                    # v runs in waves of 4 key groups; each group's accumulator
                    # gets its own full PSUM bank (512 f32) so concurrent
                    # accumulation groups never share a zero region.
                    for w0 in range(0, JG, 4):
                        nw = min(4, JG - w0)
                        psv = pa.tile([P, 4, 512], f32, tag="psv")
                        for t in range(6):
                            if w0 == 0:
                                for s0 in range(0, KP, 512):
                                    csz = min(512, KP - s0)
                                    nc.tensor.matmul(
                                        psk[:, s0 : s0 + csz],
                                        wk[:, t, 0:P],
                                        xkt[t][:, s0 : s0 + csz],
                                        start=(t == 0),
                                        stop=(t == 5),
                                    )
                                    nc.tensor.matmul(
                                        psk2[:D, s0 : s0 + csz],
                                        wk[:, t, P:192],
                                        xkt[t][:, s0 : s0 + csz],
                                        start=(t == 0),
                                        stop=(t == 5),
                                    )
                            for j in range(nw):
                                jg = w0 + j
                                nc.tensor.matmul(
                                    psv[:, j, 0:192],
                                    xkt[t][:, jg * P : (jg + 1) * P],
                                    wv[:, t, :],
                                    start=(t == 0),
                                    stop=(t == 5),
                                )
                        if w0 == 0:
                            nc.scalar.copy(kT01[:], psk[:])
                            nc.scalar.copy(kT2[:], psk2[:D, :])
                        for j in range(nw):
                            jg = w0 + j
                            for ih in range(HPC):
                                nc.vector.tensor_copy(
                                    v_sb[:, jg, ih, D : 2 * D],
                                    psv[:, j, ih * D : (ih + 1) * D],
                                )
            else:
                # generic fallback: sequential chains
                with tc.tile_pool(name="pa", bufs=2, space="PSUM") as pa:
                    for kc in range(0, KP, 1024):
                        csz = min(1024, KP - kc)
                        pk = pa.tile([P, 1024], f32, tag="g", name=f"pk_k01_{kc}")
                        for t in range(6):
                            for s0 in range(0, csz, 512):
                                ssz = min(512, csz - s0)
                                nc.tensor.matmul(
                                    pk[:, s0 : s0 + ssz],
                                    wk[:, t, 0:P],
                                    xkt[t][:, kc + s0 : kc + s0 + ssz],
                                    start=(t == 0),
                                    stop=(t == 5),
                                )
                        nc.scalar.copy(kT01[:, kc : kc + csz], pk[:, :csz])
                    for kc in range(0, KP, 1024):
                        csz = min(1024, KP - kc)
                        pk = pa.tile([P, 1024], f32, tag="g", name=f"pk_k2_{kc}")
                        for t in range(6):
                            for s0 in range(0, csz, 512):
                                ssz = min(512, csz - s0)
                                nc.tensor.matmul(
                                    pk[:D, s0 : s0 + ssz],
                                    wk[:, t, P:192],
                                    xkt[t][:, kc + s0 : kc + s0 + ssz],
                                    start=(t == 0),
                                    stop=(t == 5),
                                )
                        nc.scalar.copy(kT2[:, kc : kc + csz], pk[:D, :csz])
                    for jg in range(JG):
                        pk = pa.tile([P, 1024], f32, tag="g", name=f"pk_v_{jg}")
                        for t in range(6):
                            nc.tensor.matmul(
                                pk[:, :192],
                                xkt[t][:, jg * P : (jg + 1) * P],
                                wv[:, t, :],
                                start=(t == 0),
                                stop=(t == 5),
                            )
                        for ih in range(HPC):
                            nc.vector.tensor_copy(
                                v_sb[:, jg, ih, D : 2 * D],
                                pk[:, ih * D : (ih + 1) * D],
                            )

            # ---- A2: q01 and q2, t-outer over arriving xT tiles
            with tc.tile_pool(name="pa2", bufs=4, space="PSUM") as pa2:
                psq = [pa2.tile([P, 512], f32, tag="qA", name=f"psqA{i}") for i in range(4)]
                psq2 = [pa2.tile([P, 512], f32, tag="qB", name=f"psqB{i}") for i in range(4)]
                for t in range(6):
                    for i in range(4):
                        nc.tensor.matmul(
                            psq[i][:],
                            wq[:, t, 0:P],
                            xTt[t][:, i * 512 : (i + 1) * 512],
                            start=(t == 0),
                            stop=(t == 5),
                        )
                        nc.tensor.matmul(
                            psq2[i][:D, :],
                            wq[:, t, P:192],
                            xTt[t][:, i * 512 : (i + 1) * 512],
                            start=(t == 0),
                            stop=(t == 5),
                        )
                for i in range(4):
                    nc.scalar.copy(qT01[:, i * 512 : (i + 1) * 512], psq[i][:])
                    if i % 2 == 0:
                        nc.scalar.copy(qT2[:, i * 512 : (i + 1) * 512], psq2[i][:D, :])
                    else:
                        nc.vector.tensor_copy(qT2[:, i * 512 : (i + 1) * 512], psq2[i][:D, :])

            # ---- B + C
            with (
                tc.tile_pool(name="ps", bufs=4, space="PSUM") as pspool,
                tc.tile_pool(name="po", bufs=4, space="PSUM") as popool,
            ):
                def att_unit(ih, nh):
                    q0 = nh * 1024
                    qT_h = qT01[D * ih : D * (ih + 1), :] if ih < 2 else qT2[:, :]
                    kT_h = kT01[D * ih : D * (ih + 1), :] if ih < 2 else kT2[:, :]
                    po = [
                        popool.tile([P, 512], f32, tag="po", name=f"po{ih}{nh}{c}")
                        for c in range(2)
                    ]
                    pts = [None] * JG

                    def scores(jg):
                        pair = []
                        for c in range(2):
                            psc = pspool.tile(
                                [P, 512], f32, tag="ps", name=f"psc{ih}{nh}{jg}{c}"
                            )
                            nc.tensor.matmul(
                                psc[:],
                                kT_h[:, jg * P : (jg + 1) * P],
                                qT_h[:, q0 + c * 512 : q0 + (c + 1) * 512],
                                start=True,
                                stop=True,
                            )
                            pair.append(psc)
                        ptp = []
                        for c in range(2):
                            pt = ptpool.tile(
                                [P, 512], bf16, tag="pt", name=f"pt{ih}{nh}{jg}{c}"
                            )
                            if c == 0 or jg == 0:
                                nc.scalar.activation(
                                    pt[:], pair[c][:], Exp, scale=float(SCALE)
                                )
                            else:
                                nc.vector.tensor_scalar(
                                    pt[:].bitcast(i16),
                                    pair[c][:],
                                    float(SCH_A),
                                    float(SCH_B),
                                    Alu.mult,
                                    Alu.add,
                                )
                            ptp.append(pt)
                        if _DEBUG and ih == 0 and nh == 0 and jg == 0:
                            nc.sync.dma_start(dbg["dbg_pt"][:], ptp[0][:])
                        pts[jg] = ptp

                    def av(jg):
                        for c in range(2):
                            nc.tensor.matmul(
                                po[c][:],
                                v_sb[:, jg, ih, :],
                                pts[jg][c][:],
                                start=(jg == 0),
                                stop=(jg == JG - 1),
                            )
                        pts[jg] = None

                    # AV trails scores by TWO key groups: by av(jg) issue,
                    # exp(jg) finished ~2 PE-periods earlier, so the in-order
                    # PE queue never blocks on an exp semaphore (which would
                    # also drop the PE out of its 2.4GHz pstate).
                    scores(0)
                    scores(1)
                    for jg in range(2, JG):
                        scores(jg)
                        av(jg - 2)
                    av(JG - 2)
                    av(JG - 1)

                    if ih == 0:
                        ot_dst = OTp[0:D, :]
                    elif ih == 1:
                        ot_dst = OTp[D : 2 * D, :]
                    else:
                        ot_dst = OT2[:, :]
                    for c in range(2):
                        rb_t = rbpool.tile([P, 512], f32, tag="rb", name=f"rb{ih}{nh}{c}")
                        nc.vector.reciprocal_approx_fast(rb_t[0:D, :], po[c][0:D, :])
                        nc.vector.tensor_mul(
                            ot_dst[:, q0 + c * 512 : q0 + (c + 1) * 512],
                            po[c][D : 2 * D, :],
                            rb_t[0:D, :],
                        )
                        if _DEBUG and ih == 0 and nh == 0 and c == 0:
                            nc.scalar.copy(dbg_sb1[:], po[c][:])
                            nc.sync.dma_start(dbg["dbg_po"][:], dbg_sb1[:])

                def proj(nh, cgs):
                    q0 = nh * 1024
                    for cg in cgs:
                        for c in range(2):
                            pp = pspool.tile(
                                [P, 512], f32, tag="ps", name=f"pp{nh}{cg}{c}"
                            )
                            s0 = q0 + c * 512
                            nc.tensor.matmul(
                                pp[:],
                                p01[:, cg, :],
                                OTp[:, s0 : s0 + 512],
                                start=True,
                                stop=False,
                            )
                            nc.tensor.matmul(
                                pp[:],
                                p2[:, cg, :],
                                OT2[:, s0 : s0 + 512],
                                start=False,
                                stop=True,
                            )
                            ob = opool.tile([P, 512], bf16, name=f"ob{nh}{cg}{c}")
                            if (cg + c) % 2 == 0:
                                nc.vector.tensor_copy(ob[:], pp[:])
                            else:
                                nc.scalar.copy(ob[:], pp[:])
                            nc.sync.dma_start(
                                out_d[:, cg, s0 : s0 + 512], ob[:]
                            )

                att_unit(0, 0)
                att_unit(1, 0)
                att_unit(2, 0)
                proj(0, range(0, 3))
                att_unit(0, 1)
                proj(0, range(3, 6))
                att_unit(1, 1)
                att_unit(2, 1)
                proj(1, range(0, 6))

    nc.finalize()
    return nc


def _prep_inputs(x, mask, qkv_w, proj_w):
    """Build the 8 per-core input maps. Returns (in_maps, KP)."""
    idx = [np.nonzero(mask[b] == 0.0)[0] for b in range(B)]
    nk = max(len(i) for i in idx)
    KP = max(P, int(math.ceil(nk / P)) * P)
    JG = KP // P

    per_batch = []
    for b in range(B):
        xTb = np.ascontiguousarray(x[b].T)  # [C, N] f32
        xT_in = xTb.reshape(6, P, N).transpose(1, 0, 2).astype(BF16)
        xk = np.zeros((C, KP), np.float32)
        xk[:, : len(idx[b])] = xTb[:, idx[b]]
        xTk_in = xk.reshape(6, P, KP).transpose(1, 0, 2).astype(BF16)
        kfv = np.zeros((KP,), np.float32)
        kfv[: len(idx[b])] = 1.0
        kf_in = np.ascontiguousarray(
            np.broadcast_to(
                kfv.reshape(JG, P).T[:, :, None, None], (P, JG, HPC, D)
            )
        ).astype(BF16)
        per_batch.append((xT_in, xTk_in, kf_in))

    in_maps = []
    for c in range(NCORES):
        b, g = c // 4, c % 4
        h0 = HPC * g
        xT_in, xTk_in, kf_in = per_batch[b]
        m = {"xT": xT_in, "xTk": xTk_in, "kf": kf_in}
        for name, off in (("wqT", 0), ("wkT", C), ("wvT", 2 * C)):
            w = qkv_w[off + h0 * D : off + (h0 + HPC) * D]  # [192, C]
            m[name] = (
                np.ascontiguousarray(w.T).reshape(6, P, 192).transpose(1, 0, 2).astype(BF16)
            )
        m["pT01"] = np.ascontiguousarray(
            proj_w[:, h0 * D : (h0 + 2) * D].T
        ).reshape(P, 6, P).astype(BF16)
        m["pT2"] = np.ascontiguousarray(
            proj_w[:, (h0 + 2) * D : (h0 + 3) * D].T
        ).reshape(D, 6, P).astype(BF16)
        in_maps.append(m)
    return in_maps, KP


_CACHE = {}


def _get_program(KP):
    if KP not in _CACHE:
        _CACHE[KP] = _build_program(KP)
    return _CACHE[KP]


def kernel(x, mask, qkv_w, proj_w, proj_b, _want_results=False):
    from concourse.bass_utils import run_bass_kernel_spmd

    x = np.asarray(x, np.float32)
    mask = np.asarray(mask, np.float32)
    qkv_w = np.asarray(qkv_w, np.float32)
    proj_w = np.asarray(proj_w, np.float32)
    proj_b = np.asarray(proj_b, np.float32)

    in_maps, KP = _prep_inputs(x, mask, qkv_w, proj_w)
    nc = _get_program(KP)
    res = run_bass_kernel_spmd(nc, in_maps, list(range(NCORES)))

    out = np.empty((B, N, C), np.float32)
    for b in range(B):
        acc = None
        for c in range(4 * b, 4 * b + 4):
            a = res.results[c]["outT"]  # [128, 6, N] bf16
            a = np.asarray(a, np.float32).transpose(1, 0, 2).reshape(C, N)
            acc = a if acc is None else acc + a
        out[b] = acc.T + proj_b[None, :]
    if _want_results:
        return out, res
    return out


# revision 16
# speedup vs baseline: 1.2520x; 1.1512x over previous
"""Masked multi-head attention kernel for 8 Trainium2 NeuronCores.

Strategy (v4):
  - 24 (batch, head) pairs sharded as: core c -> batch c//4, heads [3*(c%4) .. 3*(c%4)+2].
  - Key-padding mask handled by HOST-side gather: only unmasked key positions are
    shipped/computed. Padded key slots get zeroed K columns (scores=0 -> exp=1)
    and a 0 in the indicator column of V, so they contribute nothing.
  - Softmax without max-subtraction; row-sum of exp folded into the AV matmul via
    indicator columns on V (cols 0:64 -> denominator on psum partitions 0:64,
    V values in cols 64:128 -> O on partitions 64:128).
  - exp split by 512-query chunk: chunk A -> ScalarE true exp (bf16), chunk B ->
    VectorE Schraudolph bitcast exp (int16(s*A+B) reinterpreted as bf16), so the
    two engines drain score tiles in parallel faster than PE refills them.
    First jg of each unit runs fully on ScalarE to absorb VectorE's recip/mul
    burst from the previous unit.
  - Phase-scoped PSUM pools: A1 (k01+k2+v, t-outer over arriving xTk tiles),
    A2 (q01+q2 t-outer over arriving xT tiles, 8x 1-bank slots), B/C (4 score
    slots + 4 AV-accumulator slots, all [128,512] single-bank).
  - Output projection packs heads 0,1 on a 128-partition contraction (OTp),
    head 2 separate (OT2); per-core partial written as bf16, host sums 4
    partials per batch. proj(0) split across two insertion points to spread
    its PSUM-drain copies.
  - All matmuls bf16, fp32 PSUM accumulation (fp8 blows the 2e-2 error budget).
"""

import math

import numpy as np
import ml_dtypes

BF16 = ml_dtypes.bfloat16
B, N, C = 2, 2048, 768
H = 12
D = 64
HPC = 3          # heads per core
P = 128
SCALE = D ** -0.5
NCORES = 8
LOG2E = 1.4426950408889634
SCH_A = 128.0 * LOG2E * SCALE
SCH_B = 127.0 * 128.0 - 4.5
_DEBUG = False


def _build_program(KP: int):
    from concourse import bacc, mybir
    from concourse.tile import TileContext

    JG = KP // P
    f32 = mybir.dt.float32
    bf16 = mybir.dt.bfloat16
    i16 = mybir.dt.int16
    Alu = mybir.AluOpType
    Exp = mybir.ActivationFunctionType.Exp
    nc = bacc.Bacc(None, target_bir_lowering=False)

    xT_d = nc.declare_dram_parameter("xT", [P, 6, N], bf16, False)
    xTk_d = nc.declare_dram_parameter("xTk", [P, 6, KP], bf16, False)
    kf_d = nc.declare_dram_parameter("kf", [P, JG, HPC, D], bf16, False)
    wq_d = nc.declare_dram_parameter("wqT", [P, 6, 192], bf16, False)
    wk_d = nc.declare_dram_parameter("wkT", [P, 6, 192], bf16, False)
    wv_d = nc.declare_dram_parameter("wvT", [P, 6, 192], bf16, False)
    p01_d = nc.declare_dram_parameter("pT01", [P, 6, P], bf16, False)
    p2_d = nc.declare_dram_parameter("pT2", [D, 6, P], bf16, False)
    out_d = nc.declare_dram_parameter("outT", [P, 6, N], bf16, True)

    fastA = KP <= 1024

    with TileContext(nc) as tc:
        with (
            tc.tile_pool(name="const", bufs=1) as cpool,
            tc.tile_pool(name="pt", bufs=18) as ptpool,
            tc.tile_pool(name="rb", bufs=4) as rbpool,
            tc.tile_pool(name="outp", bufs=4) as opool,
        ):
            # ---- persistent SBUF tiles
            wq = cpool.tile([P, 6, 192], bf16)
            wk = cpool.tile([P, 6, 192], bf16)
            wv = cpool.tile([P, 6, 192], bf16)
            xTt = [cpool.tile([P, N], bf16, name=f"xTt{t}") for t in range(6)]
            xkt = [cpool.tile([P, KP], bf16, name=f"xkt{t}") for t in range(6)]
            p01 = cpool.tile([P, 6, P], bf16)
            p2 = cpool.tile([D, 6, P], bf16)
            qT01 = cpool.tile([P, N], bf16)
            qT2 = cpool.tile([D, N], bf16)
            kT01 = cpool.tile([P, KP], bf16)
            kT2 = cpool.tile([D, KP], bf16)
            v_sb = cpool.tile([P, JG, HPC, 2 * D], bf16)
            OTp = cpool.tile([P, N], bf16)
            OT2 = cpool.tile([D, N], bf16)
            if _DEBUG:
                dbg = {
                    "dbg_pt": nc.declare_dram_parameter("dbg_pt", [P, 512], bf16, True),
                    "dbg_po": nc.declare_dram_parameter("dbg_po", [P, 512], f32, True),
                }
                dbg_sb1 = cpool.tile([P, 512], f32)

            # ---- input DMA. xTk first (k/v work starts earliest), then xT.
            # Two queues (sync + scalar engine) split the streams.
            nc.sync.dma_start(wk[:], wk_d[:])
            nc.scalar.dma_start(wv[:], wv_d[:])
            nc.scalar.dma_start(v_sb[:, :, :, 0:D], kf_d[:])
            for t in range(3):
                nc.sync.dma_start(xkt[t][:], xTk_d[:, t, :])
                nc.scalar.dma_start(xkt[t + 3][:], xTk_d[:, t + 3, :])
            nc.sync.dma_start(wq[:], wq_d[:])
            nc.scalar.dma_start(p01[:], p01_d[:])
            nc.scalar.dma_start(p2[:], p2_d[:])
            for t in range(3):
                nc.sync.dma_start(xTt[t][:], xT_d[:, t, :])
                nc.scalar.dma_start(xTt[t + 3][:], xT_d[:, t + 3, :])

            # ---- A1: k01, k2, v — t-outer over arriving xTk tiles
            if fastA:
                with tc.tile_pool(name="pa", bufs=1, space="PSUM") as pa:
                    psk = pa.tile([P, KP], f32, tag="psk")
                    psk2 = pa.tile([P, KP], f32, tag="psk2")
                    # v runs in waves of 4 key groups; each group's accumulator
                    # gets its own full PSUM bank (512 f32) so concurrent
                    # accumulation groups never share a zero region.
                    for w0 in range(0, JG, 4):
                        nw = min(4, JG - w0)
                        psv = pa.tile([P, 4, 512], f32, tag="psv")
                        for t in range(6):
                            if w0 == 0:
                                for s0 in range(0, KP, 512):
                                    csz = min(512, KP - s0)
                                    nc.tensor.matmul(
                                        psk[:, s0 : s0 + csz],
                                        wk[:, t, 0:P],
                                        xkt[t][:, s0 : s0 + csz],
                                        start=(t == 0),
                                        stop=(t == 5),
                                    )
                                    nc.tensor.matmul(
                                        psk2[:D, s0 : s0 + csz],
                                        wk[:, t, P:192],
                                        xkt[t][:, s0 : s0 + csz],
                                        start=(t == 0),
                                        stop=(t == 5),
                                    )
                            for j in range(nw):
                                jg = w0 + j
                                nc.tensor.matmul(
                                    psv[:, j, 0:192],
                                    xkt[t][:, jg * P : (jg + 1) * P],
                                    wv[:, t, :],
                                    start=(t == 0),
                                    stop=(t == 5),
                                )
                        if w0 == 0:
                            nc.scalar.copy(kT01[:], psk[:])
                            nc.scalar.copy(kT2[:], psk2[:D, :])
                        for j in range(nw):
                            jg = w0 + j
                            for ih in range(HPC):
                                nc.vector.tensor_copy(
                                    v_sb[:, jg, ih, D : 2 * D],
                                    psv[:, j, ih * D : (ih + 1) * D],
                                )
            else:
                # generic fallback: sequential chains
                with tc.tile_pool(name="pa", bufs=2, space="PSUM") as pa:
                    for kc in range(0, KP, 1024):
                        csz = min(1024, KP - kc)
                        pk = pa.tile([P, 1024], f32, tag="g", name=f"pk_k01_{kc}")
                        for t in range(6):
                            for s0 in range(0, csz, 512):
                                ssz = min(512, csz - s0)
                                nc.tensor.matmul(
                                    pk[:, s0 : s0 + ssz],
                                    wk[:, t, 0:P],
                                    xkt[t][:, kc + s0 : kc + s0 + ssz],
                                    start=(t == 0),
                                    stop=(t == 5),
                                )
                        nc.scalar.copy(kT01[:, kc : kc + csz], pk[:, :csz])
                    for kc in range(0, KP, 1024):
                        csz = min(1024, KP - kc)
                        pk = pa.tile([P, 1024], f32, tag="g", name=f"pk_k2_{kc}")
                        for t in range(6):
                            for s0 in range(0, csz, 512):
                                ssz = min(512, csz - s0)
                                nc.tensor.matmul(
                                    pk[:D, s0 : s0 + ssz],
                                    wk[:, t, P:192],
                                    xkt[t][:, kc + s0 : kc + s0 + ssz],
                                    start=(t == 0),
                                    stop=(t == 5),
                                )
                        nc.scalar.copy(kT2[:, kc : kc + csz], pk[:D, :csz])
                    for jg in range(JG):
                        pk = pa.tile([P, 1024], f32, tag="g", name=f"pk_v_{jg}")
                        for t in range(6):
                            nc.tensor.matmul(
                                pk[:, :192],
                                xkt[t][:, jg * P : (jg + 1) * P],
                                wv[:, t, :],
                                start=(t == 0),
                                stop=(t == 5),
                            )
                        for ih in range(HPC):
                            nc.vector.tensor_copy(
                                v_sb[:, jg, ih, D : 2 * D],
                                pk[:, ih * D : (ih + 1) * D],
                            )

            # ---- A2: q01 and q2, t-outer over arriving xT tiles
            with tc.tile_pool(name="pa2", bufs=4, space="PSUM") as pa2:
                psq = [pa2.tile([P, 512], f32, tag="qA", name=f"psqA{i}") for i in range(4)]
                psq2 = [pa2.tile([P, 512], f32, tag="qB", name=f"psqB{i}") for i in range(4)]
                for t in range(6):
                    for i in range(4):
                        nc.tensor.matmul(
                            psq[i][:],
                            wq[:, t, 0:P],
                            xTt[t][:, i * 512 : (i + 1) * 512],
                            start=(t == 0),
                            stop=(t == 5),
                        )
                        nc.tensor.matmul(
                            psq2[i][:D, :],
                            wq[:, t, P:192],
                            xTt[t][:, i * 512 : (i + 1) * 512],
                            start=(t == 0),
                            stop=(t == 5),
                        )
                for i in range(4):
                    nc.scalar.copy(qT01[:, i * 512 : (i + 1) * 512], psq[i][:])
                    if i % 2 == 0:
                        nc.scalar.copy(qT2[:, i * 512 : (i + 1) * 512], psq2[i][:D, :])
                    else:
                        nc.vector.tensor_copy(qT2[:, i * 512 : (i + 1) * 512], psq2[i][:D, :])

            # ---- B + C
            with (
                tc.tile_pool(name="ps", bufs=6, space="PSUM") as pspool,
                tc.tile_pool(name="po", bufs=2, space="PSUM") as popool,
            ):
                def att_unit(ih, nh):
                    q0 = nh * 1024
                    qT_h = qT01[D * ih : D * (ih + 1), :] if ih < 2 else qT2[:, :]
                    kT_h = kT01[D * ih : D * (ih + 1), :] if ih < 2 else kT2[:, :]
                    po = [
                        popool.tile([P, 512], f32, tag="po", name=f"po{ih}{nh}{c}")
                        for c in range(2)
                    ]
                    pts = [None] * JG

                    def scores(jg):
                        pair = []
                        for c in range(2):
                            psc = pspool.tile(
                                [P, 512], f32, tag="ps", name=f"psc{ih}{nh}{jg}{c}"
                            )
                            nc.tensor.matmul(
                                psc[:],
                                kT_h[:, jg * P : (jg + 1) * P],
                                qT_h[:, q0 + c * 512 : q0 + (c + 1) * 512],
                                start=True,
                                stop=True,
                            )
                            pair.append(psc)
                        ptp = []
                        for c in range(2):
                            pt = ptpool.tile(
                                [P, 512], bf16, tag="pt", name=f"pt{ih}{nh}{jg}{c}"
                            )
                            if c == 0 or jg == 0:
                                nc.scalar.activation(
                                    pt[:], pair[c][:], Exp, scale=float(SCALE)
                                )
                            else:
                                nc.vector.tensor_scalar(
                                    pt[:].bitcast(i16),
                                    pair[c][:],
                                    float(SCH_A),
                                    float(SCH_B),
                                    Alu.mult,
                                    Alu.add,
                                )
                            ptp.append(pt)
                        if _DEBUG and ih == 0 and nh == 0 and jg == 0:
                            nc.sync.dma_start(dbg["dbg_pt"][:], ptp[0][:])
                        pts[jg] = ptp

                    def av(jg):
                        for c in range(2):
                            nc.tensor.matmul(
                                po[c][:],
                                v_sb[:, jg, ih, :],
                                pts[jg][c][:],
                                start=(jg == 0),
                                stop=(jg == JG - 1),
                            )
                        pts[jg] = None

                    # All scores first (6 PSUM slots deep, so slot-reuse
                    # waits trail by 3 key groups and the PE never blocks),
                    # then AV as ONE interleaved accumulation mega-chain:
                    # 2*JG matmuls, each stationary loaded once, no PSUM
                    # drain until the stop — keeps the PE array streaming
                    # at its top pstate.
                    for jg in range(JG):
                        scores(jg)
                    for jg in range(JG):
                        av(jg)

                    if ih == 0:
                        ot_dst = OTp[0:D, :]
                    elif ih == 1:
                        ot_dst = OTp[D : 2 * D, :]
                    else:
                        ot_dst = OT2[:, :]
                    for c in range(2):
                        rb_t = rbpool.tile([P, 512], f32, tag="rb", name=f"rb{ih}{nh}{c}")
                        nc.vector.reciprocal_approx_fast(rb_t[0:D, :], po[c][0:D, :])
                        nc.vector.tensor_mul(
                            ot_dst[:, q0 + c * 512 : q0 + (c + 1) * 512],
                            po[c][D : 2 * D, :],
                            rb_t[0:D, :],
                        )
                        if _DEBUG and ih == 0 and nh == 0 and c == 0:
                            nc.scalar.copy(dbg_sb1[:], po[c][:])
                            nc.sync.dma_start(dbg["dbg_po"][:], dbg_sb1[:])

                def proj(nh, cgs):
                    q0 = nh * 1024
                    for cg in cgs:
                        for c in range(2):
                            pp = pspool.tile(
                                [P, 512], f32, tag="ps", name=f"pp{nh}{cg}{c}"
                            )
                            s0 = q0 + c * 512
                            nc.tensor.matmul(
                                pp[:],
                                p01[:, cg, :],
                                OTp[:, s0 : s0 + 512],
                                start=True,
                                stop=False,
                            )
                            nc.tensor.matmul(
                                pp[:],
                                p2[:, cg, :],
                                OT2[:, s0 : s0 + 512],
                                start=False,
                                stop=True,
                            )
                            ob = opool.tile([P, 512], bf16, name=f"ob{nh}{cg}{c}")
                            if (cg + c) % 2 == 0:
                                nc.vector.tensor_copy(ob[:], pp[:])
                            else:
                                nc.scalar.copy(ob[:], pp[:])
                            nc.sync.dma_start(
                                out_d[:, cg, s0 : s0 + 512], ob[:]
                            )

                att_unit(0, 0)
                att_unit(1, 0)
                att_unit(2, 0)
                proj(0, range(0, 3))
                att_unit(0, 1)
                proj(0, range(3, 6))
                att_unit(1, 1)
                att_unit(2, 1)
                proj(1, range(0, 6))

    nc.finalize()
    return nc


def _prep_inputs(x, mask, qkv_w, proj_w):
    """Build the 8 per-core input maps. Returns (in_maps, KP)."""
    idx = [np.nonzero(mask[b] == 0.0)[0] for b in range(B)]
    nk = max(len(i) for i in idx)
    KP = max(P, int(math.ceil(nk / P)) * P)
    JG = KP // P

    per_batch = []
    for b in range(B):
        xTb = np.ascontiguousarray(x[b].T)  # [C, N] f32
        xT_in = xTb.reshape(6, P, N).transpose(1, 0, 2).astype(BF16)
        xk = np.zeros((C, KP), np.float32)
        xk[:, : len(idx[b])] = xTb[:, idx[b]]
        xTk_in = xk.reshape(6, P, KP).transpose(1, 0, 2).astype(BF16)
        kfv = np.zeros((KP,), np.float32)
        kfv[: len(idx[b])] = 1.0
        kf_in = np.ascontiguousarray(
            np.broadcast_to(
                kfv.reshape(JG, P).T[:, :, None, None], (P, JG, HPC, D)
            )
        ).astype(BF16)
        per_batch.append((xT_in, xTk_in, kf_in))

    in_maps = []
    for c in range(NCORES):
        b, g = c // 4, c % 4
        h0 = HPC * g
        xT_in, xTk_in, kf_in = per_batch[b]
        m = {"xT": xT_in, "xTk": xTk_in, "kf": kf_in}
        for name, off in (("wqT", 0), ("wkT", C), ("wvT", 2 * C)):
            w = qkv_w[off + h0 * D : off + (h0 + HPC) * D]  # [192, C]
            m[name] = (
                np.ascontiguousarray(w.T).reshape(6, P, 192).transpose(1, 0, 2).astype(BF16)
            )
        m["pT01"] = np.ascontiguousarray(
            proj_w[:, h0 * D : (h0 + 2) * D].T
        ).reshape(P, 6, P).astype(BF16)
        m["pT2"] = np.ascontiguousarray(
            proj_w[:, (h0 + 2) * D : (h0 + 3) * D].T
        ).reshape(D, 6, P).astype(BF16)
        in_maps.append(m)
    return in_maps, KP


_CACHE = {}


def _get_program(KP):
    if KP not in _CACHE:
        _CACHE[KP] = _build_program(KP)
    return _CACHE[KP]


def kernel(x, mask, qkv_w, proj_w, proj_b, _want_results=False):
    from concourse.bass_utils import run_bass_kernel_spmd

    x = np.asarray(x, np.float32)
    mask = np.asarray(mask, np.float32)
    qkv_w = np.asarray(qkv_w, np.float32)
    proj_w = np.asarray(proj_w, np.float32)
    proj_b = np.asarray(proj_b, np.float32)

    in_maps, KP = _prep_inputs(x, mask, qkv_w, proj_w)
    nc = _get_program(KP)
    res = run_bass_kernel_spmd(nc, in_maps, list(range(NCORES)))

    out = np.empty((B, N, C), np.float32)
    for b in range(B):
        acc = None
        for c in range(4 * b, 4 * b + 4):
            a = res.results[c]["outT"]  # [128, 6, N] bf16
            a = np.asarray(a, np.float32).transpose(1, 0, 2).reshape(C, N)
            acc = a if acc is None else acc + a
        out[b] = acc.T + proj_b[None, :]
    if _want_results:
        return out, res
    return out


# revision 17
# speedup vs baseline: 1.3553x; 1.0825x over previous
"""Masked multi-head attention kernel for 8 Trainium2 NeuronCores.

Strategy (v4):
  - 24 (batch, head) pairs sharded as: core c -> batch c//4, heads [3*(c%4) .. 3*(c%4)+2].
  - Key-padding mask handled by HOST-side gather: only unmasked key positions are
    shipped/computed. Padded key slots get zeroed K columns (scores=0 -> exp=1)
    and a 0 in the indicator column of V, so they contribute nothing.
  - Softmax without max-subtraction; row-sum of exp folded into the AV matmul via
    indicator columns on V (cols 0:64 -> denominator on psum partitions 0:64,
    V values in cols 64:128 -> O on partitions 64:128).
  - exp split by 512-query chunk: chunk A -> ScalarE true exp (bf16), chunk B ->
    VectorE Schraudolph bitcast exp (int16(s*A+B) reinterpreted as bf16), so the
    two engines drain score tiles in parallel faster than PE refills them.
    First jg of each unit runs fully on ScalarE to absorb VectorE's recip/mul
    burst from the previous unit.
  - Phase-scoped PSUM pools: A1 (k01+k2+v, t-outer over arriving xTk tiles),
    A2 (q01+q2 t-outer over arriving xT tiles, 8x 1-bank slots), B/C (4 score
    slots + 4 AV-accumulator slots, all [128,512] single-bank).
  - Output projection packs heads 0,1 on a 128-partition contraction (OTp),
    head 2 separate (OT2); per-core partial written as bf16, host sums 4
    partials per batch. proj(0) split across two insertion points to spread
    its PSUM-drain copies.
  - All matmuls bf16, fp32 PSUM accumulation (fp8 blows the 2e-2 error budget).
"""

import math

import numpy as np
import ml_dtypes

BF16 = ml_dtypes.bfloat16
B, N, C = 2, 2048, 768
H = 12
D = 64
HPC = 3          # heads per core
P = 128
SCALE = D ** -0.5
NCORES = 8
LOG2E = 1.4426950408889634
SCH_A = 128.0 * LOG2E * SCALE
SCH_B = 127.0 * 128.0 - 4.5
_DEBUG = False


def _build_program(KP: int):
    from concourse import bacc, mybir
    from concourse.tile import TileContext

    JG = KP // P
    f32 = mybir.dt.float32
    bf16 = mybir.dt.bfloat16
    i16 = mybir.dt.int16
    Alu = mybir.AluOpType
    Exp = mybir.ActivationFunctionType.Exp
    nc = bacc.Bacc(None, target_bir_lowering=False)

    xT_d = nc.declare_dram_parameter("xT", [P, 6, N], bf16, False)
    xTk_d = nc.declare_dram_parameter("xTk", [P, 6, KP], bf16, False)
    kf_d = nc.declare_dram_parameter("kf", [P, JG, HPC, D], bf16, False)
    wq_d = nc.declare_dram_parameter("wqT", [P, 6, 192], bf16, False)
    wk_d = nc.declare_dram_parameter("wkT", [P, 6, 192], bf16, False)
    wv_d = nc.declare_dram_parameter("wvT", [P, 6, 192], bf16, False)
    p01_d = nc.declare_dram_parameter("pT01", [P, 6, P], bf16, False)
    p2_d = nc.declare_dram_parameter("pT2", [D, 6, P], bf16, False)
    out_d = nc.declare_dram_parameter("outT", [P, 6, N], bf16, True)

    fastA = KP <= 1024

    with TileContext(nc) as tc:
        with (
            tc.tile_pool(name="const", bufs=1) as cpool,
            tc.tile_pool(name="pt", bufs=18) as ptpool,
            tc.tile_pool(name="rb", bufs=4) as rbpool,
            tc.tile_pool(name="outp", bufs=4) as opool,
        ):
            # ---- persistent SBUF tiles
            wq = cpool.tile([P, 6, 192], bf16)
            wk = cpool.tile([P, 6, 192], bf16)
            wv = cpool.tile([P, 6, 192], bf16)
            xTt = [cpool.tile([P, N], bf16, name=f"xTt{t}") for t in range(6)]
            xkt = [cpool.tile([P, KP], bf16, name=f"xkt{t}") for t in range(6)]
            p01 = cpool.tile([P, 6, P], bf16)
            p2 = cpool.tile([D, 6, P], bf16)
            qT01 = cpool.tile([P, N], bf16)
            qT2 = cpool.tile([D, N], bf16)
            kT01 = cpool.tile([P, KP], bf16)
            kT2 = cpool.tile([D, KP], bf16)
            v_sb = cpool.tile([P, JG, HPC, 2 * D], bf16)
            OTp = cpool.tile([P, N], bf16)
            OT2 = cpool.tile([D, N], bf16)
            if _DEBUG:
                dbg = {
                    "dbg_pt": nc.declare_dram_parameter("dbg_pt", [P, 512], bf16, True),
                    "dbg_po": nc.declare_dram_parameter("dbg_po", [P, 512], f32, True),
                }
                dbg_sb1 = cpool.tile([P, 512], f32)

            # ---- input DMA in exact consumption order on the sync queue
            # (per-t xTk then xT pairs feed the merged k+q t-loop); small
            # non-critical tensors ride the scalar-engine queue.
            nc.sync.dma_start(wk[:], wk_d[:])
            nc.sync.dma_start(wq[:], wq_d[:])
            nc.scalar.dma_start(wv[:], wv_d[:])
            nc.scalar.dma_start(v_sb[:, :, :, 0:D], kf_d[:])
            for t in range(6):
                nc.sync.dma_start(xkt[t][:], xTk_d[:, t, :])
                nc.sync.dma_start(xTt[t][:], xT_d[:, t, :])
            nc.scalar.dma_start(p01[:], p01_d[:])
            nc.scalar.dma_start(p2[:], p2_d[:])

            # ---- A (fast path): ONE t-outer loop for k01+k2+q01 chasing the
            # per-t DMA arrivals (exactly 8 PSUM banks), then v waves + q2
            # overlapped. Generic fallback for KP>1024 keeps the simple shape.
            if fastA:
                with (
                    tc.tile_pool(name="pamk", bufs=1, space="PSUM") as pamk,
                    tc.tile_pool(name="pamq", bufs=4, space="PSUM") as pamq,
                ):
                    psk = pamk.tile([P, KP], f32, tag="psk")
                    psk2 = pamk.tile([P, KP], f32, tag="psk2")
                    psq = [pamq.tile([P, 512], f32, tag="qA", name=f"psqA{i}") for i in range(4)]
                    for t in range(6):
                        for s0 in range(0, KP, 512):
                            csz = min(512, KP - s0)
                            nc.tensor.matmul(
                                psk[:, s0 : s0 + csz],
                                wk[:, t, 0:P],
                                xkt[t][:, s0 : s0 + csz],
                                start=(t == 0),
                                stop=(t == 5),
                            )
                            nc.tensor.matmul(
                                psk2[:D, s0 : s0 + csz],
                                wk[:, t, P:192],
                                xkt[t][:, s0 : s0 + csz],
                                start=(t == 0),
                                stop=(t == 5),
                            )
                        for i in range(4):
                            nc.tensor.matmul(
                                psq[i][:],
                                wq[:, t, 0:P],
                                xTt[t][:, i * 512 : (i + 1) * 512],
                                start=(t == 0),
                                stop=(t == 5),
                            )
                    # copy order: first-unit needs qT01 chunks 0,1 and kT01
                    nc.scalar.copy(qT01[:, 0:512], psq[0][:])
                    nc.scalar.copy(qT01[:, 512:1024], psq[1][:])
                    nc.scalar.copy(kT01[:], psk[:])
                    nc.scalar.copy(qT01[:, 1024:1536], psq[2][:])
                    nc.scalar.copy(qT01[:, 1536:2048], psq[3][:])
                    nc.scalar.copy(kT2[:], psk2[:D, :])
                with (
                    tc.tile_pool(name="pav", bufs=1, space="PSUM") as pav,
                    tc.tile_pool(name="paq2", bufs=4, space="PSUM") as paq2,
                ):
                    psq2 = [paq2.tile([P, 512], f32, tag="qB", name=f"psqB{i}") for i in range(4)]

                    def vwave(w0):
                        nw = min(4, JG - w0)
                        psv = pav.tile([P, 4, 512], f32, tag="psv")
                        for t in range(6):
                            for j in range(nw):
                                jg = w0 + j
                                nc.tensor.matmul(
                                    psv[:, j, 0:192],
                                    xkt[t][:, jg * P : (jg + 1) * P],
                                    wv[:, t, :],
                                    start=(t == 0),
                                    stop=(t == 5),
                                )
                        for j in range(nw):
                            jg = w0 + j
                            for ih in range(HPC):
                                nc.vector.tensor_copy(
                                    v_sb[:, jg, ih, D : 2 * D],
                                    psv[:, j, ih * D : (ih + 1) * D],
                                )

                    vwave(0)
                    for t in range(6):
                        for i in range(4):
                            nc.tensor.matmul(
                                psq2[i][:D, :],
                                wq[:, t, P:192],
                                xTt[t][:, i * 512 : (i + 1) * 512],
                                start=(t == 0),
                                stop=(t == 5),
                            )
                    vwave(4)
                    for i in range(4):
                        if i % 2 == 0:
                            nc.scalar.copy(qT2[:, i * 512 : (i + 1) * 512], psq2[i][:D, :])
                        else:
                            nc.vector.tensor_copy(qT2[:, i * 512 : (i + 1) * 512], psq2[i][:D, :])
            else:
                # generic fallback: sequential chains
                with tc.tile_pool(name="pa", bufs=2, space="PSUM") as pa:
                    for kc in range(0, KP, 1024):
                        csz = min(1024, KP - kc)
                        pk = pa.tile([P, 1024], f32, tag="g", name=f"pk_k01_{kc}")
                        for t in range(6):
                            for s0 in range(0, csz, 512):
                                ssz = min(512, csz - s0)
                                nc.tensor.matmul(
                                    pk[:, s0 : s0 + ssz],
                                    wk[:, t, 0:P],
                                    xkt[t][:, kc + s0 : kc + s0 + ssz],
                                    start=(t == 0),
                                    stop=(t == 5),
                                )
                        nc.scalar.copy(kT01[:, kc : kc + csz], pk[:, :csz])
                    for kc in range(0, KP, 1024):
                        csz = min(1024, KP - kc)
                        pk = pa.tile([P, 1024], f32, tag="g", name=f"pk_k2_{kc}")
                        for t in range(6):
                            for s0 in range(0, csz, 512):
                                ssz = min(512, csz - s0)
                                nc.tensor.matmul(
                                    pk[:D, s0 : s0 + ssz],
                                    wk[:, t, P:192],
                                    xkt[t][:, kc + s0 : kc + s0 + ssz],
                                    start=(t == 0),
                                    stop=(t == 5),
                                )
                        nc.scalar.copy(kT2[:, kc : kc + csz], pk[:D, :csz])
                    for jg in range(JG):
                        pk = pa.tile([P, 1024], f32, tag="g", name=f"pk_v_{jg}")
                        for t in range(6):
                            nc.tensor.matmul(
                                pk[:, :192],
                                xkt[t][:, jg * P : (jg + 1) * P],
                                wv[:, t, :],
                                start=(t == 0),
                                stop=(t == 5),
                            )
                        for ih in range(HPC):
                            nc.vector.tensor_copy(
                                v_sb[:, jg, ih, D : 2 * D],
                                pk[:, ih * D : (ih + 1) * D],
                            )
                    for half in range(2):
                        pk = pa.tile([P, 1024], f32, tag="g", name=f"pk_q01_{half}")
                        for t in range(6):
                            for s0 in (0, 512):
                                nc.tensor.matmul(
                                    pk[:, s0 : s0 + 512],
                                    wq[:, t, 0:P],
                                    xTt[t][:, half * 1024 + s0 : half * 1024 + s0 + 512],
                                    start=(t == 0),
                                    stop=(t == 5),
                                )
                        nc.scalar.copy(qT01[:, half * 1024 : half * 1024 + 1024], pk[:])
                    for half in range(2):
                        pk = pa.tile([P, 1024], f32, tag="g", name=f"pk_q2_{half}")
                        for t in range(6):
                            for s0 in (0, 512):
                                nc.tensor.matmul(
                                    pk[:D, s0 : s0 + 512],
                                    wq[:, t, P:192],
                                    xTt[t][:, half * 1024 + s0 : half * 1024 + s0 + 512],
                                    start=(t == 0),
                                    stop=(t == 5),
                                )
                        nc.scalar.copy(qT2[:, half * 1024 : half * 1024 + 1024], pk[:D, :])

            # ---- B + C
            with (
                tc.tile_pool(name="ps", bufs=6, space="PSUM") as pspool,
                tc.tile_pool(name="po", bufs=2, space="PSUM") as popool,
            ):
                def att_unit(ih, nh):
                    q0 = nh * 1024
                    qT_h = qT01[D * ih : D * (ih + 1), :] if ih < 2 else qT2[:, :]
                    kT_h = kT01[D * ih : D * (ih + 1), :] if ih < 2 else kT2[:, :]
                    po = [
                        popool.tile([P, 512], f32, tag="po", name=f"po{ih}{nh}{c}")
                        for c in range(2)
                    ]
                    pts = [None] * JG

                    def scores(jg):
                        pair = []
                        for c in range(2):
                            psc = pspool.tile(
                                [P, 512], f32, tag="ps", name=f"psc{ih}{nh}{jg}{c}"
                            )
                            nc.tensor.matmul(
                                psc[:],
                                kT_h[:, jg * P : (jg + 1) * P],
                                qT_h[:, q0 + c * 512 : q0 + (c + 1) * 512],
                                start=True,
                                stop=True,
                            )
                            pair.append(psc)
                        ptp = []
                        for c in range(2):
                            pt = ptpool.tile(
                                [P, 512], bf16, tag="pt", name=f"pt{ih}{nh}{jg}{c}"
                            )
                            if c == 0 or jg == 0:
                                nc.scalar.activation(
                                    pt[:], pair[c][:], Exp, scale=float(SCALE)
                                )
                            else:
                                nc.vector.tensor_scalar(
                                    pt[:].bitcast(i16),
                                    pair[c][:],
                                    float(SCH_A),
                                    float(SCH_B),
                                    Alu.mult,
                                    Alu.add,
                                )
                            ptp.append(pt)
                        if _DEBUG and ih == 0 and nh == 0 and jg == 0:
                            nc.sync.dma_start(dbg["dbg_pt"][:], ptp[0][:])
                        pts[jg] = ptp

                    def av(jg):
                        for c in range(2):
                            nc.tensor.matmul(
                                po[c][:],
                                v_sb[:, jg, ih, :],
                                pts[jg][c][:],
                                start=(jg == 0),
                                stop=(jg == JG - 1),
                            )
                        pts[jg] = None

                    # All scores first (6 PSUM slots deep, so slot-reuse
                    # waits trail by 3 key groups and the PE never blocks),
                    # then AV as ONE interleaved accumulation mega-chain:
                    # 2*JG matmuls, each stationary loaded once, no PSUM
                    # drain until the stop — keeps the PE array streaming
                    # at its top pstate.
                    for jg in range(JG):
                        scores(jg)
                    for jg in range(JG):
                        av(jg)

                    if ih == 0:
                        ot_dst = OTp[0:D, :]
                    elif ih == 1:
                        ot_dst = OTp[D : 2 * D, :]
                    else:
                        ot_dst = OT2[:, :]
                    for c in range(2):
                        rb_t = rbpool.tile([P, 512], f32, tag="rb", name=f"rb{ih}{nh}{c}")
                        nc.vector.reciprocal_approx_fast(rb_t[0:D, :], po[c][0:D, :])
                        nc.vector.tensor_mul(
                            ot_dst[:, q0 + c * 512 : q0 + (c + 1) * 512],
                            po[c][D : 2 * D, :],
                            rb_t[0:D, :],
                        )
                        if _DEBUG and ih == 0 and nh == 0 and c == 0:
                            nc.scalar.copy(dbg_sb1[:], po[c][:])
                            nc.sync.dma_start(dbg["dbg_po"][:], dbg_sb1[:])

                def proj(nh, cgs):
                    q0 = nh * 1024
                    for cg in cgs:
                        for c in range(2):
                            pp = pspool.tile(
                                [P, 512], f32, tag="ps", name=f"pp{nh}{cg}{c}"
                            )
                            s0 = q0 + c * 512
                            nc.tensor.matmul(
                                pp[:],
                                p01[:, cg, :],
                                OTp[:, s0 : s0 + 512],
                                start=True,
                                stop=False,
                            )
                            nc.tensor.matmul(
                                pp[:],
                                p2[:, cg, :],
                                OT2[:, s0 : s0 + 512],
                                start=False,
                                stop=True,
                            )
                            ob = opool.tile([P, 512], bf16, name=f"ob{nh}{cg}{c}")
                            if (cg + c) % 2 == 0:
                                nc.vector.tensor_copy(ob[:], pp[:])
                            else:
                                nc.scalar.copy(ob[:], pp[:])
                            nc.sync.dma_start(
                                out_d[:, cg, s0 : s0 + 512], ob[:]
                            )

                att_unit(0, 0)
                att_unit(1, 0)
                att_unit(2, 0)
                proj(0, range(0, 3))
                att_unit(0, 1)
                proj(0, range(3, 6))
                att_unit(1, 1)
                att_unit(2, 1)
                proj(1, range(0, 6))

    nc.finalize()
    return nc


def _prep_inputs(x, mask, qkv_w, proj_w):
    """Build the 8 per-core input maps. Returns (in_maps, KP)."""
    idx = [np.nonzero(mask[b] == 0.0)[0] for b in range(B)]
    nk = max(len(i) for i in idx)
    KP = max(P, int(math.ceil(nk / P)) * P)
    JG = KP // P

    per_batch = []
    for b in range(B):
        xTb = np.ascontiguousarray(x[b].T)  # [C, N] f32
        xT_in = xTb.reshape(6, P, N).transpose(1, 0, 2).astype(BF16)
        xk = np.zeros((C, KP), np.float32)
        xk[:, : len(idx[b])] = xTb[:, idx[b]]
        xTk_in = xk.reshape(6, P, KP).transpose(1, 0, 2).astype(BF16)
        kfv = np.zeros((KP,), np.float32)
        kfv[: len(idx[b])] = 1.0
        kf_in = np.ascontiguousarray(
            np.broadcast_to(
                kfv.reshape(JG, P).T[:, :, None, None], (P, JG, HPC, D)
            )
        ).astype(BF16)
        per_batch.append((xT_in, xTk_in, kf_in))

    in_maps = []
    for c in range(NCORES):
        b, g = c // 4, c % 4
        h0 = HPC * g
        xT_in, xTk_in, kf_in = per_batch[b]
        m = {"xT": xT_in, "xTk": xTk_in, "kf": kf_in}
        for name, off in (("wqT", 0), ("wkT", C), ("wvT", 2 * C)):
            w = qkv_w[off + h0 * D : off + (h0 + HPC) * D]  # [192, C]
            m[name] = (
                np.ascontiguousarray(w.T).reshape(6, P, 192).transpose(1, 0, 2).astype(BF16)
            )
        m["pT01"] = np.ascontiguousarray(
            proj_w[:, h0 * D : (h0 + 2) * D].T
        ).reshape(P, 6, P).astype(BF16)
        m["pT2"] = np.ascontiguousarray(
            proj_w[:, (h0 + 2) * D : (h0 + 3) * D].T
        ).reshape(D, 6, P).astype(BF16)
        in_maps.append(m)
    return in_maps, KP


_CACHE = {}


def _get_program(KP):
    if KP not in _CACHE:
        _CACHE[KP] = _build_program(KP)
    return _CACHE[KP]


def kernel(x, mask, qkv_w, proj_w, proj_b, _want_results=False):
    from concourse.bass_utils import run_bass_kernel_spmd

    x = np.asarray(x, np.float32)
    mask = np.asarray(mask, np.float32)
    qkv_w = np.asarray(qkv_w, np.float32)
    proj_w = np.asarray(proj_w, np.float32)
    proj_b = np.asarray(proj_b, np.float32)

    in_maps, KP = _prep_inputs(x, mask, qkv_w, proj_w)
    nc = _get_program(KP)
    res = run_bass_kernel_spmd(nc, in_maps, list(range(NCORES)))

    out = np.empty((B, N, C), np.float32)
    for b in range(B):
        acc = None
        for c in range(4 * b, 4 * b + 4):
            a = res.results[c]["outT"]  # [128, 6, N] bf16
            a = np.asarray(a, np.float32).transpose(1, 0, 2).reshape(C, N)
            acc = a if acc is None else acc + a
        out[b] = acc.T + proj_b[None, :]
    if _want_results:
        return out, res
    return out
